# revision 2
# baseline (speedup 1.0000x reference)
"""AxialBlock kernel for 8 trn2 NeuronCores.

Strategy: data-parallel over batch N=16 (2 images per core), executed on the
8 NeuronCores through the JAX/PJRT neuron backend as a single pmap'd program.
Every train-mode BatchNorm needs full-batch statistics, so the per-device
batch stats are combined with jax.lax.pmean inside the pmap (an 8-way
allreduce of a few hundred floats — the only cross-core traffic).

Shapes are hardcoded per the problem spec: x [16,128,64,64] f32, MID=128,
COUT=256, G=8 groups, GP=16 planes per group, K=64.
"""

import os

os.environ.setdefault("JAX_PLATFORMS", "axon,cpu")

import numpy as np
import jax
import jax.numpy as jnp
from functools import partial

NB, CIN, COUT, MID, G, K = 16, 128, 256, 128, 8, 64
GP = MID // G  # 16
EPS = 1e-5
NCORES = 8
NLOC = NB // NCORES  # 2 images per core

_WEIGHT_NAMES = [
    'c1_w', 'c1_b', 'cd_w', 'cd_b', 'bn1_g', 'bn1_b',
    'h_qkv_w', 'h_bq_g', 'h_bq_b', 'h_bs_g', 'h_bs_b', 'h_bo_g', 'h_bo_b',
    'w_qkv_w', 'w_bq_g', 'w_bq_b', 'w_bs_g', 'w_bs_b', 'w_bo_g', 'w_bo_b',
    'cu_w', 'cu_b', 'bn2_g', 'bn2_b',
    'h_emb', 'w_emb',  # host-precomputed rel embedding lookups [2*GP, K, K]
]


def _bn_dp(x, g, b, axes):
    """Train-mode batchnorm with cross-device (data-parallel) statistics.

    Channel axis is 1; `axes` are the reduction axes.  Statistics combine
    exactly across the 8 equal shards via pmean of (mean, mean-of-squares).
    """
    m_loc = jnp.mean(x, axes, keepdims=True)
    m2_loc = jnp.mean(x * x, axes, keepdims=True)
    m = jax.lax.pmean(m_loc, 'dp')
    m2 = jax.lax.pmean(m2_loc, 'dp')
    v = m2 - m * m
    sh = (1, -1) + (1,) * (x.ndim - 2)
    return g.reshape(sh) * (x - m) * jax.lax.rsqrt(v + EPS) + b.reshape(sh)


def _axial_dp(x, qkv_w, bq_g, bq_b, bs_g, bs_b, bo_g, bo_b, emb):
    # x: [B_local, MID, K]; attention along last axis; emb [2*GP, K, K]
    B = x.shape[0]
    qkv = _bn_dp(jnp.einsum('oc,bcl->bol', qkv_w, x), bq_g, bq_b, (0, 2))
    qkv = qkv.reshape(B, G, 2 * GP, K)
    q, k, v = qkv[:, :, :GP // 2], qkv[:, :, GP // 2:GP], qkv[:, :, GP:]
    q_e, k_e, v_e = emb[:GP // 2], emb[GP // 2:GP], emb[GP:]
    qr = 0.1 * jnp.einsum('bgci,cij->bgij', q, q_e)
    kr = 0.1 * jnp.einsum('bgci,cij->bgji', k, k_e)
    qk = jnp.einsum('bgci,bgcj->bgij', q, k)
    sim = jnp.concatenate([qk, qr, kr], axis=1)            # [B, 3G, K, K]
    sim = _bn_dp(sim, bs_g, bs_b, (0, 2, 3)).reshape(B, 3, G, K, K).sum(1)
    sim = jax.nn.softmax(sim, axis=-1)                     # [B, G, K, K]
    sv = jnp.einsum('bgij,bgcj->bgci', sim, v)
    sve = 0.1 * jnp.einsum('bgij,cij->bgci', sim, v_e)
    out = jnp.concatenate([sv, sve], axis=1).reshape(B, 2 * MID, K)
    out = _bn_dp(out, bo_g, bo_b, (0, 2))
    return out.reshape(B, MID, 2, K).sum(2)                # [B, MID, K]


def _block_dp(x, w):
    # x: [NLOC, CIN, K, K] (this device's batch shard); w: dict of weights
    x_out = (jnp.einsum('oc,nchw->nohw', w['c1_w'], x)
             + w['c1_b'][None, :, None, None])
    out = (jnp.einsum('oc,nchw->nohw', w['cd_w'], x)
           + w['cd_b'][None, :, None, None])
    out = jax.nn.relu(_bn_dp(out, w['bn1_g'], w['bn1_b'], (0, 2, 3)))
    n = out.shape[0]
    # height block: sequences along H, batch (n, w)
    h_in = out.transpose(0, 3, 1, 2).reshape(n * K, MID, K)
    h = _axial_dp(h_in, w['h_qkv_w'], w['h_bq_g'], w['h_bq_b'],
                  w['h_bs_g'], w['h_bs_b'], w['h_bo_g'], w['h_bo_b'],
                  w['h_emb'])
    h = h.reshape(n, K, MID, K).transpose(0, 2, 3, 1)      # [n, C, H, W]
    # width block: sequences along W, batch (n, h)
    w_in = h.transpose(0, 2, 1, 3).reshape(n * K, MID, K)
    wo = _axial_dp(w_in, w['w_qkv_w'], w['w_bq_g'], w['w_bq_b'],
                   w['w_bs_g'], w['w_bs_b'], w['w_bo_g'], w['w_bo_b'],
                   w['w_emb'])
    wo = wo.reshape(n, K, MID, K).transpose(0, 2, 1, 3)    # [n, C, H, W]
    out = (jnp.einsum('oc,nchw->nohw', w['cu_w'], wo)
           + w['cu_b'][None, :, None, None])
    out = _bn_dp(out, w['bn2_g'], w['bn2_b'], (0, 2, 3))
    return out + x_out


_pmapped = None


def _get_pmapped():
    global _pmapped
    if _pmapped is None:
        _pmapped = jax.pmap(_block_dp, axis_name='dp',
                            in_axes=(0, None), devices=jax.devices()[:NCORES])
    return _pmapped


def kernel(**inputs):
    inp = {kk: np.asarray(vv, dtype=np.float32) for kk, vv in inputs.items()}

    # Host-side: the relative-position embedding lookup is a pure gather on a
    # [2*GP, 2K-1] table shared by every core — precompute emb [2*GP, K, K].
    idx = np.arange(K)[:, None] - np.arange(K)[None, :] + K - 1
    weights = {}
    for name in _WEIGHT_NAMES:
        if name.endswith('_emb'):
            weights[name] = jnp.asarray(
                np.ascontiguousarray(inp[name[:2] + 'rel'][:, idx]))
        else:
            weights[name] = jnp.asarray(inp[name])

    x_sharded = jnp.asarray(
        inp['x'].reshape(NCORES, NLOC, CIN, K, K))

    out = _get_pmapped()(x_sharded, weights)
    out = np.asarray(out, dtype=np.float32).reshape(NB, COUT, K, K)
    return out
